# revision 8
# baseline (speedup 1.0000x reference)
"""Trainium2 Bass kernel for AQT-style int8 fake-quant matmul.

Computes: out = fq(lhs) @ fq(rhs), where fq is symmetric int8 fake-quant
(per-row abs-max scale for lhs, per-column for rhs):
    s_l = rowmax(|lhs|)/127 ; q_l = round(lhs/s_l) ; fq(lhs) = q_l*s_l
    out[m,n] = sum_k q_l[m,k]*q_r[k,n] * s_l[m] * s_r[n]

Strategy (8 NeuronCores, no collectives):
  - rhs is column-sharded [4096, 2048] per core; lhs replicated.
  - Host pre-transposes lhs to a strip-tiled layout (pure layout prep; all
    math happens on-device): H[s, p, kt, m] = lhs[128*s+m, 128*kt+p], so each
    128-column strip of lhs^T DMAs as one contiguous 2 MB block with 16 KiB
    contiguous per partition.
  - On device, per core:
      Phase R: stream rhs shard, abs-max over k (free-axis chain + partition
        fold), build r_r=127/amax and s_r=amax/127 broadcasts, re-stream rhs
        and quantize to *exact integer* bf16 (magic-number round).
      Phase L (per 128-row strip of lhs): DMA strip, abs-max over k, quantize
        to integer bf16, 128 matmuls (4 psum banks x 32 k-tiles) against the
        resident q_rhs, then evict psum with s_l (per-partition ACT scale)
        and s_r (VectorE multiply) folded in.
  - Integer q values (<=127) are exact in bf16 and products accumulate in
    fp32 PSUM, so the only deviation from the f32 reference is accumulation
    order (~1e-6 relative).
"""

import numpy as np

P = 128
MM_N = 512  # matmul moving free dim == one PSUM bank of fp32
MAGIC = 12582912.0  # 1.5 * 2**23: fp32 round-to-nearest-even magic constant

# Full problem shape (hardcoded per harness contract).
FULL_M, FULL_K, FULL_N = 8192, 4096, 16384
N_CORES = 8


def _build(M, K, N_shard, o_bufs=4, small_bufs=3):
    """Build the single-core Bass graph; same graph runs SPMD on all cores."""
    import concourse.bass as bass  # noqa: F401  (registers engines)
    import concourse.tile as tile
    from concourse import bacc, bass_isa, mybir

    f32 = mybir.dt.float32
    bf16 = mybir.dt.bfloat16
    ALU = mybir.AluOpType
    ACTF = mybir.ActivationFunctionType
    AX = mybir.AxisListType

    n_strips = M // P
    n_kt = K // P
    n_n4 = N_shard // MM_N
    inv127 = 1.0 / 127.0

    nc = bacc.Bacc(
        "TRN2", debug=False, dynamic_dma_scratch_size=8192,
        enable_partition_id=False,
    )
    lhsT_d = nc.dram_tensor(
        "lhsT", [n_strips, P, n_kt, P], f32, kind="ExternalInput"
    ).ap()
    rhs_d = nc.dram_tensor("rhs", [K, N_shard], f32, kind="ExternalInput").ap()
    out_d = nc.dram_tensor("out", [M, N_shard], f32, kind="ExternalOutput").ap()

    with tile.TileContext(nc) as tc:
        with (
            tc.tile_pool(name="f32buf", bufs=2) as f32pool,
            tc.tile_pool(name="qrhs", bufs=1) as qrhs_pool,
            tc.tile_pool(name="lq", bufs=2) as lq_pool,
            tc.tile_pool(name="accp", bufs=1) as acc_pool,
            tc.tile_pool(name="srp", bufs=1) as sr_pool,
            tc.tile_pool(name="rrp", bufs=1) as rr_pool,
            tc.tile_pool(name="small", bufs=small_bufs) as small_pool,
            tc.tile_pool(name="otile", bufs=o_bufs) as o_pool,
            tc.tile_pool(name="psum", bufs=8, space="PSUM") as psum_pool,
        ):
            # ---------------- Phase R: quantize the rhs shard ----------------
            # (walrus codegen lacks AluOpType.abs_max, so take |x| via an
            # int32 bitcast sign-bit mask, then fold with plain max)
            i32 = mybir.dt.int32
            acc = acc_pool.tile([P, N_shard], f32, name="acc")
            for kt in range(n_kt):
                rt = f32pool.tile([P, N_shard], f32, tag="f32buf", name="rt")
                nc.sync.dma_start(rt, rhs_d[kt * P : (kt + 1) * P, :])
                nc.vector.tensor_scalar(
                    out=rt.bitcast(i32), in0=rt.bitcast(i32),
                    scalar1=0x7FFFFFFF, scalar2=None, op0=ALU.bitwise_and,
                )
                if kt == 0:
                    nc.vector.tensor_copy(out=acc, in_=rt)
                else:
                    nc.vector.tensor_tensor(out=acc, in0=acc, in1=rt, op=ALU.max)
            # sr <- amax all-reduced (and broadcast) across partitions
            sr = sr_pool.tile([P, N_shard], f32, name="sr")
            nc.gpsimd.partition_all_reduce(
                sr, acc, channels=P, reduce_op=bass_isa.ReduceOp.absmax
            )
            nc.vector.tensor_scalar(
                out=sr, in0=sr, scalar1=1e-30, scalar2=None, op0=ALU.max
            )
            # rr = 127/amax ; sr = amax/127
            rr = rr_pool.tile([P, N_shard], f32, name="rr")
            nc.vector.reciprocal(rr, sr)
            nc.vector.tensor_scalar(
                out=rr, in0=rr, scalar1=127.0, scalar2=None, op0=ALU.mult
            )
            nc.vector.tensor_scalar(
                out=sr, in0=sr, scalar1=inv127, scalar2=None, op0=ALU.mult
            )
            # pass 2: re-stream rhs and quantize to integer bf16
            qrhs = qrhs_pool.tile([P, n_kt, N_shard], bf16, name="qrhs")
            for kt in range(n_kt):
                rt = f32pool.tile([P, N_shard], f32, tag="f32buf", name="rt")
                nc.sync.dma_start(rt, rhs_d[kt * P : (kt + 1) * P, :])
                nc.vector.tensor_tensor(out=rt, in0=rt, in1=rr, op=ALU.mult)
                nc.vector.tensor_scalar(
                    out=qrhs[:, kt, :], in0=rt, scalar1=MAGIC, scalar2=MAGIC,
                    op0=ALU.add, op1=ALU.subtract,
                )

            # ---------------- Phase L: per-strip quant + matmul ----------------
            for s in range(n_strips):
                lf = f32pool.tile([P, n_kt * P], f32, tag="f32buf", name="lf")
                nc.sync.dma_start(lf, lhsT_d[s].rearrange("p kt m -> p (kt m)"))
                lf3 = lf.rearrange("p (kt m) -> p kt m", kt=n_kt)
                # abs-max over kt (free) then over partitions
                pam = small_pool.tile([P, P], f32, name="pam")
                nc.vector.tensor_reduce(
                    out=pam,
                    in_=lf.rearrange("p (kt m) -> p m kt", kt=n_kt),
                    axis=AX.X,
                    op=ALU.max,
                    apply_absolute_value=True,
                )
                amx = small_pool.tile([P, P], f32, name="amx")
                nc.gpsimd.partition_all_reduce(
                    amx, pam, channels=P, reduce_op=bass_isa.ReduceOp.absmax
                )
                nc.vector.tensor_scalar(
                    out=amx, in0=amx, scalar1=1e-30, scalar2=None, op0=ALU.max
                )
                # s_l as a per-partition column via 32x32 transposes of the
                # (row-replicated) amax tile
                sd = small_pool.tile([P, 32], f32, name="sd")
                for b in range(P // 32):
                    nc.vector.transpose(
                        sd[32 * b : 32 * (b + 1), :], amx[0:32, 32 * b : 32 * (b + 1)]
                    )
                slc = small_pool.tile([P, 1], f32, name="slc")
                nc.vector.tensor_scalar(
                    out=slc, in0=sd[:, 0:1], scalar1=inv127, scalar2=None,
                    op0=ALU.mult,
                )
                # r = 127/amax broadcast tile
                rb = small_pool.tile([P, P], f32, name="rb")
                nc.vector.reciprocal(rb, amx)
                nc.vector.tensor_scalar(
                    out=rb, in0=rb, scalar1=127.0, scalar2=None, op0=ALU.mult
                )
                # quantize strip: lf *= r (k-tile broadcast), then magic round
                nc.vector.tensor_tensor(
                    out=lf3, in0=lf3,
                    in1=rb.unsqueeze(1).broadcast_to([P, n_kt, P]),
                    op=ALU.mult,
                )
                lq = lq_pool.tile([P, n_kt * P], bf16, name="lq")
                nc.vector.tensor_scalar(
                    out=lq, in0=lf, scalar1=MAGIC, scalar2=MAGIC,
                    op0=ALU.add, op1=ALU.subtract,
                )
                lq3 = lq.rearrange("p (kt m) -> p kt m", kt=n_kt)
                # matmuls: 4 psum banks x 32 k-tiles
                for n4 in range(n_n4):
                    ps = psum_pool.tile([P, MM_N], f32, name="ps")
                    for kt in range(n_kt):
                        nc.tensor.matmul(
                            ps,
                            lq3[:, kt, :],
                            qrhs[:, kt, n4 * MM_N : (n4 + 1) * MM_N],
                            start=(kt == 0),
                            stop=(kt == n_kt - 1),
                        )
                    o = o_pool.tile([P, MM_N], f32, name="o")
                    nc.scalar.activation(o, ps, ACTF.Copy, scale=slc)
                    nc.vector.tensor_tensor(
                        out=o, in0=o, in1=sr[:, n4 * MM_N : (n4 + 1) * MM_N],
                        op=ALU.mult,
                    )
                    nc.scalar.dma_start(
                        out_d[s * P : (s + 1) * P, n4 * MM_N : (n4 + 1) * MM_N], o
                    )

    nc.compile()
    return nc


def _host_pack_lhsT(lhs):
    """lhs [M, K] f32 -> H[s, p, kt, m] = lhs[128*s+m, 128*kt+p], contiguous."""
    M, K = lhs.shape
    l4 = lhs.reshape(M // P, P, K // P, P)  # [s, m, kt, p]
    return np.ascontiguousarray(l4.transpose(0, 3, 2, 1))


_NC_CACHE = {}


def _get_nc(M, K, N_shard):
    key = (M, K, N_shard)
    if key not in _NC_CACHE:
        _NC_CACHE[key] = _build(M, K, N_shard)
    return _NC_CACHE[key]


def kernel(lhs, rhs, _trace=False):
    """Full-input entry point: shards across 8 cores, returns full output."""
    from concourse.bass_utils import run_bass_kernel_spmd

    lhs = np.asarray(lhs, dtype=np.float32)
    rhs = np.asarray(rhs, dtype=np.float32)
    M, K = lhs.shape
    K2, N = rhs.shape
    assert K == K2
    n_shard = N // N_CORES

    H = _host_pack_lhsT(lhs)
    in_maps = [
        {
            "lhsT": H,
            "rhs": np.ascontiguousarray(rhs[:, c * n_shard : (c + 1) * n_shard]),
        }
        for c in range(N_CORES)
    ]
    nc = _get_nc(M, K, n_shard)
    res = run_bass_kernel_spmd(
        nc, in_maps, core_ids=list(range(N_CORES)), trace=_trace
    )
    out = np.concatenate([r["out"] for r in res.results], axis=1)
    if _trace:
        kernel.last_exec_time_ns = res.exec_time_ns
        kernel.last_results = res
    return out
